# revision 4
# baseline (speedup 1.0000x reference)
"""LBP extractor on 8 Trainium2 NeuronCores — v3 (engine-balanced).

See kernel2 docstring for the core scheme (fp16 bucket quantization,
complement trick, fp8e5 odd-byte plane views, DoubleRow assembly).

v3 additions (cost-model balancing; the DVE was the v2 bottleneck):
  * UV mega-tile: both halo row copies in one SBUF tensor so one DVE
    tensor_tensor with a [(pair, 2), (1, 2049)] access pattern computes two
    planes ({A, D}) in a single op at the 2x rate.
  * Plane C is computed as Pool subtract (diff = U - V) followed by either
    - DVE tensor_scalar is_ge vs 0 (4x mode, 0/1 plane), or
    - ACT Sign(diff + 2^-15) (a +-1 plane consumed with halved DR weights
      and a -30 constant adjustment),
    chosen per tile to balance DVE vs ACT occupancy (SIGN_FRAC).
  * One 2048-wide PSUM mega-tile (4 banks) per chunk: 16 DoubleRow matmuls
    accumulate into 512-col slices; a single wide ACT copy (+bias) converts
    to uint8.
"""

import math

import numpy as np

H = 8192
W = 8192
NCORES = 8
RPC = H // NCORES

CW = 2048
SUB = 512
TRO = 127
PCW = CW + 4

QBITS_BASE = 11264
QBITS_SCALE = 20479.0 / 256.0
SIGN_EPS = 2.0**-15

A0, B0, C0, D0 = 0, PCW, 2 * PCW, 3 * PCW

# fraction of tile-units whose C-plane bit extraction runs on ACT (Sign)
SIGN_NUM, SIGN_DEN = 4, 5

# debug switches (bisect aids)
PAIR_AD = True  # paired {A, D} compare op
POOL_DIFF = True  # C via Pool subtract (else DVE is_ge directly)
WIDE_PSUM = True  # [128, 2048] 4-bank psum + single wide ACT copy

# DoubleRow matmul table: (rhs_plane_elem_off, rhs_col_off, delta, w0/sh0, w1/sh1)
# indices 0..3 for is_ge-C tiles; index 4 replaces 1 on sign-C tiles.
DRS = [
    (A0, 0, 2 * 1, (1, 0), (-16, 1)),  # A direct | A' derived (rhs: A[m], A[m+1])
    (C0, 0, 2 * 1, (-64, 1), (4, 0)),  # C' | C  (0/1 plane)
    (B0, 0, 2 * (D0 - B0) + 2, (2, 0), (-8, 0)),  # B | D' (rhs: B[m], D[m+1])
    (B0, 0, 2 * (D0 - B0), (-32, 1), (128, 0)),  # B' | D
    (C0, 0, 2 * 1, (-32, 1), (2, 0)),  # C' | C  (+-1 plane, halved)
]
BIAS_ISGE = 120.0  # sum of derived weights
BIAS_SIGN = 120.0 - 30.0  # C direct 4b = 2s+2; C' -64b' = -32s'-32 -> -30


def _build_bass(h, w, rpc, cw):
    import concourse.bacc as bacc
    import concourse.bass as bass
    import concourse.mybir as mybir
    from concourse.tile import TileContext

    f16 = mybir.dt.float16
    f32 = mybir.dt.float32
    fp8e4 = mybir.dt.float8e4
    fp8e5 = mybir.dt.float8e5
    u8 = mybir.dt.uint8

    pcw = cw + 4
    w2 = w + 2
    n_tiles = math.ceil(rpc / TRO)
    n_chunks = w // cw
    n_sub = cw // SUB

    nc = bacc.Bacc("TRN2", target_bir_lowering=False)
    x = nc.dram_tensor("x", [rpc + 2, w2], f16, kind="ExternalInput")
    wident = nc.dram_tensor("wident", [128, 5, 2, 128], fp8e4, kind="ExternalInput")
    y = nc.dram_tensor("y", [rpc, w], u8, kind="ExternalOutput")

    def rap(base_ap, elem_off, dims):
        return bass.AP(
            tensor=base_ap.tensor, offset=base_ap.offset + elem_off, ap=dims
        )

    with TileContext(nc) as tc:
        with (
            tc.tile_pool(name="const", bufs=1) as cpool,
            tc.tile_pool(name="img", bufs=2) as ipool,
            tc.tile_pool(name="diff", bufs=2) as dpool,
            tc.tile_pool(name="planes", bufs=2) as ppool,
            tc.tile_pool(name="outb", bufs=3) as opool,
            tc.tile_pool(name="psum", bufs=2 if WIDE_PSUM else 8, space="PSUM") as qpool,
        ):
            wt = cpool.tile([128, 5, 2, 128], fp8e4)
            nc.sync.dma_start(wt[:, :, :, :], wident[:, :, :, :])
            eps_t = cpool.tile([128, 1], f32)
            nc.vector.memset(eps_t[:, :], SIGN_EPS)

            unit = 0
            for t in range(n_tiles):
                r0 = t * TRO
                nrows = min(TRO, rpc - r0)
                k = nrows + 1
                uv = ipool.tile([128, 2 * w2], f16, tag="uv")
                nc.sync.dma_start(uv[0:k, 0:w2], x[r0 : r0 + k, :])
                nc.sync.dma_start(uv[0:k, w2 : 2 * w2], x[r0 + 1 : r0 + 1 + k, :])
                uva = uv[:, :]
                pstr = uva.ap[0][0]
                for q in range(n_chunks):
                    cb = q * cw
                    use_sign = (unit % SIGN_DEN) < SIGN_NUM
                    unit += 1
                    pl = ppool.tile([128, 4 * pcw], f16, tag="pl")
                    pla = pl[:, :]
                    ppstr = pla.ap[0][0]
                    # pair op {A, D}: in0 = (U[m], V[m]), in1 = V[m+1] x2
                    if PAIR_AD:
                        nc.vector.tensor_tensor(
                            out=rap(pla, A0, [[ppstr, k], [D0 - A0, 2], [1, cw + 1]]),
                            in0=rap(uva, cb, [[pstr, k], [w2, 2], [1, cw + 1]]),
                            in1=rap(
                                uva, w2 + cb + 1, [[pstr, k], [0, 2], [1, cw + 1]]
                            ),
                            op=mybir.AluOpType.is_ge,
                        )
                    else:
                        nc.vector.tensor_tensor(
                            out=pl[0:k, A0 : A0 + cw + 1],
                            in0=rap(uva, cb, [[pstr, k], [1, cw + 1]]),
                            in1=rap(uva, w2 + cb + 1, [[pstr, k], [1, cw + 1]]),
                            op=mybir.AluOpType.is_ge,
                        )
                        nc.vector.tensor_tensor(
                            out=pl[0:k, D0 : D0 + cw + 1],
                            in0=rap(uva, w2 + cb, [[pstr, k], [1, cw + 1]]),
                            in1=rap(uva, w2 + cb + 1, [[pstr, k], [1, cw + 1]]),
                            op=mybir.AluOpType.is_ge,
                        )
                    # B = U[m+1] >= V[m+1]
                    nc.vector.tensor_tensor(
                        out=pl[0:k, B0 : B0 + cw + 1],
                        in0=rap(uva, cb + 1, [[pstr, k], [1, cw + 1]]),
                        in1=rap(uva, w2 + cb + 1, [[pstr, k], [1, cw + 1]]),
                        op=mybir.AluOpType.is_ge,
                    )
                    # C: Pool diff = U[m+1] - V[m], then bit-extract
                    if POOL_DIFF:
                        df = dpool.tile([128, pcw], f16, tag="df")
                        nc.gpsimd.tensor_tensor(
                            out=df[0:k, 0 : cw + 1],
                            in0=rap(uva, cb + 1, [[pstr, k], [1, cw + 1]]),
                            in1=rap(uva, w2 + cb, [[pstr, k], [1, cw + 1]]),
                            op=mybir.AluOpType.subtract,
                        )
                        if use_sign:
                            nc.scalar.sign(
                                pl[0:k, C0 : C0 + cw + 1],
                                df[0:k, 0 : cw + 1],
                                bias=eps_t[0:k, :],
                            )
                        else:
                            nc.vector.tensor_scalar(
                                out=pl[0:k, C0 : C0 + cw + 1],
                                in0=df[0:k, 0 : cw + 1],
                                scalar1=0.0,
                                scalar2=None,
                                op0=mybir.AluOpType.is_ge,
                            )
                    else:
                        use_sign = False
                        nc.vector.tensor_tensor(
                            out=pl[0:k, C0 : C0 + cw + 1],
                            in0=rap(uva, cb + 1, [[pstr, k], [1, cw + 1]]),
                            in1=rap(uva, w2 + cb, [[pstr, k], [1, cw + 1]]),
                            op=mybir.AluOpType.is_ge,
                        )
                    pl8 = pla.bitcast(fp8e5)
                    p8str = pl8.ap[0][0]
                    ou = opool.tile([128, cw], u8, tag="ou")
                    bias = BIAS_SIGN if use_sign else BIAS_ISGE
                    pss = []
                    for sq in range(n_sub):
                        if WIDE_PSUM:
                            if sq == 0:
                                ps = qpool.tile([128, cw], f32, tag="ps")
                                pss.append(ps)
                            psv = ps[:, sq * SUB : (sq + 1) * SUB]
                        else:
                            ps = qpool.tile([128, SUB], f32, tag="ps")
                            pss.append(ps)
                            psv = ps[:, :]
                        for j in range(4):
                            d = 4 if (use_sign and j == 1) else j
                            p0, coff, delta, _s0, _s1 = DRS[d]
                            rhs = rap(
                                pl8,
                                2 * (p0 + coff + SUB * sq) + 1,
                                [[p8str, k], [delta, 2], [2, SUB]],
                            )
                            nc.tensor.matmul(
                                psv,
                                lhsT=wt[0:k, d, :, :],
                                rhs=rhs,
                                start=(j == 0),
                                stop=(j == 3),
                                perf_mode=mybir.MatmulPerfMode.DoubleRow,
                                skip_group_check=True,
                            )
                    if WIDE_PSUM:
                        nc.scalar.activation(
                            ou[0:nrows, :],
                            pss[0][0:nrows, :],
                            mybir.ActivationFunctionType.Copy,
                            bias=bias,
                        )
                    else:
                        for sq in range(n_sub):
                            nc.scalar.activation(
                                ou[0:nrows, sq * SUB : (sq + 1) * SUB],
                                pss[sq][0:nrows, :],
                                mybir.ActivationFunctionType.Copy,
                                bias=bias,
                            )
                    nc.sync.dma_start(
                        y[r0 : r0 + nrows, cb : cb + cw], ou[0:nrows, :]
                    )

    nc.compile()
    return nc


def _make_wident():
    import ml_dtypes

    wf = np.zeros((128, 5, 2, 128), np.float32)
    idx = np.arange(128)
    for d, (_p0, _c, _delta, s0, s1) in enumerate(DRS):
        for s, (wgt, sh) in enumerate((s0, s1)):
            if sh == 0:
                wf[idx, d, s, idx] = float(wgt)
            else:
                wf[idx[1:], d, s, idx[:-1]] = float(wgt)
    return wf.astype(ml_dtypes.float8_e4m3fn)


def quantize(img):
    bits = (QBITS_BASE + np.floor(np.asarray(img, np.float32) * QBITS_SCALE)).astype(
        np.uint16
    )
    return bits.view(np.float16)


def _host_inputs(img, h, w, rpc, ncores):
    q = quantize(img)
    pad = np.full((h + 2, w + 2), np.float16(-1.0), np.float16)
    pad[1 : h + 1, 1 : w + 1] = q
    pad[0, 1 : w + 1] = q[h - 1]
    pad[1 : h + 1, 0] = q[:, w - 1]
    pad[0, 0] = q[h - 1, w - 1]

    wid = _make_wident()
    in_maps = []
    for c in range(ncores):
        in_maps.append(
            {
                "x": np.ascontiguousarray(pad[rpc * c : rpc * c + rpc + 2, :]),
                "wident": wid,
            }
        )
    return in_maps


_NC_CACHE = None


def _get_nc():
    global _NC_CACHE
    if _NC_CACHE is None:
        _NC_CACHE = _build_bass(H, W, RPC, CW)
    return _NC_CACHE


def kernel(rgb_image: np.ndarray, _trace: bool = False, _tmpdir: str | None = None):
    from concourse import bass_utils

    img = np.asarray(rgb_image, dtype=np.float32)
    assert img.shape == (H, W), img.shape
    in_maps = _host_inputs(img, H, W, RPC, NCORES)
    nc = _get_nc()
    try:
        res = bass_utils.run_bass_kernel_spmd(
            nc,
            in_maps,
            core_ids=list(range(NCORES)),
            trace=_trace,
            tmpdir=_tmpdir,
        )
    except ModuleNotFoundError:
        res = bass_utils.run_bass_kernel_spmd(
            nc, in_maps, core_ids=list(range(NCORES)), trace=False
        )
    out = np.concatenate([r["y"] for r in res.results], axis=0)
    if _trace:
        kernel.last_results = res
    return out


# revision 5
# speedup vs baseline: 1.2475x; 1.2475x over previous
"""LBP extractor on 8 Trainium2 NeuronCores — v3 (engine-balanced).

See kernel2 docstring for the core scheme (fp16 bucket quantization,
complement trick, fp8e5 odd-byte plane views, DoubleRow assembly).

v3 additions (cost-model balancing; the DVE was the v2 bottleneck):
  * UV mega-tile: both halo row copies in one SBUF tensor so one DVE
    tensor_tensor with a [(pair, 2), (1, 2049)] access pattern computes two
    planes ({A, D}) in a single op at the 2x rate.
  * Plane C is computed as Pool subtract (diff = U - V) followed by either
    - DVE tensor_scalar is_ge vs 0 (4x mode, 0/1 plane), or
    - ACT Sign(diff + 2^-15) (a +-1 plane consumed with halved DR weights
      and a -30 constant adjustment),
    chosen per tile to balance DVE vs ACT occupancy (SIGN_FRAC).
  * One 2048-wide PSUM mega-tile (4 banks) per chunk: 16 DoubleRow matmuls
    accumulate into 512-col slices; a single wide ACT copy (+bias) converts
    to uint8.
"""

import math

import numpy as np

H = 8192
W = 8192
NCORES = 8
RPC = H // NCORES

CW = 2048
SUB = 512
TRO = 127
PCW = CW + 4

QBITS_BASE = 11264
QBITS_SCALE = 20479.0 / 256.0
SIGN_EPS = 2.0**-15

A0, B0, C0, D0 = 0, PCW, 2 * PCW, 3 * PCW

# fraction of tile-units whose C-plane bit extraction runs on ACT (Sign)
SIGN_NUM, SIGN_DEN = 4, 5

# debug switches (bisect aids)
PAIR_AD = True  # paired {A, D} compare op
POOL_DIFF = True  # C via Pool subtract (else DVE is_ge directly)
WIDE_PSUM = False  # [128, 2048] 4-bank psum + single wide ACT copy
PIPE_DIFF = False  # issue the Pool subtract one unit ahead (software pipeline)
OUT_DMA_ACT = False  # issue output DMAs from the ACT queue (SP prefetches inputs)
CHUNK_DMA = True  # per-chunk input DMAs (fast pipeline start)
CPW2 = 2052  # per-chunk uv tile: U half width (V half at same offset +CPW2)
DVE_C_EVERY = 10  # every Nth unit computes C directly on DVE (Pool/DVE balance)

# DoubleRow matmul table: (rhs_plane_elem_off, rhs_col_off, delta, w0/sh0, w1/sh1)
# indices 0..3 for is_ge-C tiles; index 4 replaces 1 on sign-C tiles.
DRS = [
    (A0, 0, 2 * 1, (1, 0), (-16, 1)),  # A direct | A' derived (rhs: A[m], A[m+1])
    (C0, 0, 2 * 1, (-64, 1), (4, 0)),  # C' | C  (0/1 plane)
    (B0, 0, 2 * (D0 - B0) + 2, (2, 0), (-8, 0)),  # B | D' (rhs: B[m], D[m+1])
    (B0, 0, 2 * (D0 - B0), (-32, 1), (128, 0)),  # B' | D
    (C0, 0, 2 * 1, (-32, 1), (2, 0)),  # C' | C  (+-1 plane, halved)
]
BIAS_ISGE = 120.0  # sum of derived weights
BIAS_SIGN = 120.0 - 30.0  # C direct 4b = 2s+2; C' -64b' = -32s'-32 -> -30


def _build_bass(h, w, rpc, cw):
    import concourse.bacc as bacc
    import concourse.bass as bass
    import concourse.mybir as mybir
    from concourse.tile import TileContext

    f16 = mybir.dt.float16
    f32 = mybir.dt.float32
    fp8e4 = mybir.dt.float8e4
    fp8e5 = mybir.dt.float8e5
    u8 = mybir.dt.uint8

    pcw = cw + 4
    w2 = w + 2
    n_tiles = math.ceil(rpc / TRO)
    n_chunks = w // cw
    n_sub = cw // SUB

    nc = bacc.Bacc("TRN2", target_bir_lowering=False)
    x = nc.dram_tensor("x", [rpc + 2, w2], f16, kind="ExternalInput")
    wident = nc.dram_tensor("wident", [128, 5, 2, 128], fp8e4, kind="ExternalInput")
    y = nc.dram_tensor("y", [rpc, w], u8, kind="ExternalOutput")

    def rap(base_ap, elem_off, dims):
        return bass.AP(
            tensor=base_ap.tensor, offset=base_ap.offset + elem_off, ap=dims
        )

    with TileContext(nc) as tc:
        with (
            tc.tile_pool(name="const", bufs=1) as cpool,
            tc.tile_pool(name="img", bufs=6) as ipool,
            tc.tile_pool(name="diff", bufs=3) as dpool,
            tc.tile_pool(name="planes", bufs=3) as ppool,
            tc.tile_pool(name="outb", bufs=3) as opool,
            tc.tile_pool(name="psum", bufs=2 if WIDE_PSUM else 8, space="PSUM") as qpool,
        ):
            wt = cpool.tile([128, 5, 2, 128], fp8e4)
            nc.sync.dma_start(wt[:, :, :, :], wident[:, :, :, :])
            eps_t = cpool.tile([128, 1], f32)
            nc.vector.memset(eps_t[:, :], SIGN_EPS)

            units = [(t, q) for t in range(n_tiles) for q in range(n_chunks)]
            if len(units) > n_chunks:
                # run the cheap ragged last row-tile first: its tiny DMAs fill
                # the pipe instantly and the tail then ends on a regular unit
                units = units[-n_chunks:] + units[:-n_chunks]
            uv_tiles = {}
            df_tiles = {}
            cpw2 = CPW2

            def tile_rows(t):
                r0 = t * TRO
                nrows = min(TRO, rpc - r0)
                return r0, nrows, nrows + 1

            def ensure_uv(t, q):
                if (t, q) not in uv_tiles:
                    r0, _nrows, k = tile_rows(t)
                    c0 = q * cw
                    cwd = min(cpw2, w2 - c0)
                    uv = ipool.tile(
                        [128, 2 * cpw2], f16, tag="uv", name=f"uv{t}_{q}"
                    )
                    nc.sync.dma_start(
                        uv[0:k, 0:cwd], x[r0 : r0 + k, c0 : c0 + cwd]
                    )
                    nc.sync.dma_start(
                        uv[0:k, cpw2 : cpw2 + cwd],
                        x[r0 + 1 : r0 + 1 + k, c0 : c0 + cwd],
                    )
                    uv_tiles[(t, q)] = uv
                return uv_tiles[(t, q)]

            def issue_sub(u):
                if not POOL_DIFF or u >= len(units):
                    return
                if DVE_C_EVERY and (u % DVE_C_EVERY) == (DVE_C_EVERY - 1):
                    return  # this unit's C runs directly on DVE
                t, q = units[u]
                _r0, _nrows, k = tile_rows(t)
                uva = ensure_uv(t, q)[:, :]
                pstr = uva.ap[0][0]
                df = dpool.tile([128, pcw], f16, tag="df", name=f"df{u}")
                nc.gpsimd.tensor_tensor(
                    out=df[0:k, 0 : cw + 1],
                    in0=rap(uva, 1, [[pstr, k], [1, cw + 1]]),
                    in1=rap(uva, cpw2, [[pstr, k], [1, cw + 1]]),
                    op=mybir.AluOpType.subtract,
                )
                df_tiles[u] = df

            if PIPE_DIFF:
                issue_sub(0)
            for unit, (t, q) in enumerate(units):
                r0, nrows, k = tile_rows(t)
                uv = ensure_uv(t, q)
                uva = uv[:, :]
                pstr = uva.ap[0][0]
                if PIPE_DIFF:
                    issue_sub(unit + 1)
                else:
                    issue_sub(unit)
                cb = q * cw
                use_sign = (unit % SIGN_DEN) < SIGN_NUM
                if True:
                    pl = ppool.tile([128, 4 * pcw], f16, tag="pl")
                    pla = pl[:, :]
                    ppstr = pla.ap[0][0]
                    # pair op {A, D}: in0 = (U[m], V[m]), in1 = V[m+1] x2
                    if PAIR_AD:
                        nc.vector.tensor_tensor(
                            out=rap(pla, A0, [[ppstr, k], [D0 - A0, 2], [1, cw + 1]]),
                            in0=rap(uva, 0, [[pstr, k], [cpw2, 2], [1, cw + 1]]),
                            in1=rap(
                                uva, cpw2 + 1, [[pstr, k], [0, 2], [1, cw + 1]]
                            ),
                            op=mybir.AluOpType.is_ge,
                        )
                    else:
                        nc.vector.tensor_tensor(
                            out=pl[0:k, A0 : A0 + cw + 1],
                            in0=rap(uva, 0, [[pstr, k], [1, cw + 1]]),
                            in1=rap(uva, cpw2 + 1, [[pstr, k], [1, cw + 1]]),
                            op=mybir.AluOpType.is_ge,
                        )
                        nc.vector.tensor_tensor(
                            out=pl[0:k, D0 : D0 + cw + 1],
                            in0=rap(uva, cpw2, [[pstr, k], [1, cw + 1]]),
                            in1=rap(uva, cpw2 + 1, [[pstr, k], [1, cw + 1]]),
                            op=mybir.AluOpType.is_ge,
                        )
                    # B = U[m+1] >= V[m+1]
                    nc.vector.tensor_tensor(
                        out=pl[0:k, B0 : B0 + cw + 1],
                        in0=rap(uva, 1, [[pstr, k], [1, cw + 1]]),
                        in1=rap(uva, cpw2 + 1, [[pstr, k], [1, cw + 1]]),
                        op=mybir.AluOpType.is_ge,
                    )
                    # C: Pool diff = U[m+1] - V[m] (issued earlier), bit-extract
                    if POOL_DIFF and unit not in df_tiles and DVE_C_EVERY:
                        use_sign = False
                        nc.vector.tensor_tensor(
                            out=pl[0:k, C0 : C0 + cw + 1],
                            in0=rap(uva, 1, [[pstr, k], [1, cw + 1]]),
                            in1=rap(uva, cpw2, [[pstr, k], [1, cw + 1]]),
                            op=mybir.AluOpType.is_ge,
                        )
                    elif POOL_DIFF:
                        df = df_tiles.pop(unit)
                        if use_sign:
                            nc.scalar.sign(
                                pl[0:k, C0 : C0 + cw + 1],
                                df[0:k, 0 : cw + 1],
                                bias=eps_t[0:k, :],
                            )
                        else:
                            nc.vector.tensor_scalar(
                                out=pl[0:k, C0 : C0 + cw + 1],
                                in0=df[0:k, 0 : cw + 1],
                                scalar1=0.0,
                                scalar2=None,
                                op0=mybir.AluOpType.is_ge,
                            )
                    else:
                        use_sign = False
                        nc.vector.tensor_tensor(
                            out=pl[0:k, C0 : C0 + cw + 1],
                            in0=rap(uva, 1, [[pstr, k], [1, cw + 1]]),
                            in1=rap(uva, cpw2, [[pstr, k], [1, cw + 1]]),
                            op=mybir.AluOpType.is_ge,
                        )
                    pl8 = pla.bitcast(fp8e5)
                    p8str = pl8.ap[0][0]
                    ou = opool.tile([128, cw], u8, tag="ou")
                    bias = BIAS_SIGN if use_sign else BIAS_ISGE
                    pss = []
                    for sq in range(n_sub):
                        if WIDE_PSUM:
                            if sq == 0:
                                ps = qpool.tile([128, cw], f32, tag="ps")
                                pss.append(ps)
                            psv = ps[:, sq * SUB : (sq + 1) * SUB]
                        else:
                            ps = qpool.tile([128, SUB], f32, tag="ps")
                            pss.append(ps)
                            psv = ps[:, :]
                        for j in range(4):
                            d = 4 if (use_sign and j == 1) else j
                            p0, coff, delta, _s0, _s1 = DRS[d]
                            rhs = rap(
                                pl8,
                                2 * (p0 + coff + SUB * sq) + 1,
                                [[p8str, k], [delta, 2], [2, SUB]],
                            )
                            nc.tensor.matmul(
                                psv,
                                lhsT=wt[0:k, d, :, :],
                                rhs=rhs,
                                start=(j == 0),
                                stop=(j == 3),
                                perf_mode=mybir.MatmulPerfMode.DoubleRow,
                                skip_group_check=True,
                            )
                    if WIDE_PSUM:
                        nc.scalar.activation(
                            ou[0:nrows, :],
                            pss[0][0:nrows, :],
                            mybir.ActivationFunctionType.Copy,
                            bias=bias,
                        )
                    else:
                        for sq in range(n_sub):
                            nc.scalar.activation(
                                ou[0:nrows, sq * SUB : (sq + 1) * SUB],
                                pss[sq][0:nrows, :],
                                mybir.ActivationFunctionType.Copy,
                                bias=bias,
                            )
                    out_eng = nc.scalar if OUT_DMA_ACT else nc.sync
                    out_eng.dma_start(
                        y[r0 : r0 + nrows, cb : cb + cw], ou[0:nrows, :]
                    )

    nc.compile()
    return nc


def _make_wident():
    import ml_dtypes

    wf = np.zeros((128, 5, 2, 128), np.float32)
    idx = np.arange(128)
    for d, (_p0, _c, _delta, s0, s1) in enumerate(DRS):
        for s, (wgt, sh) in enumerate((s0, s1)):
            if sh == 0:
                wf[idx, d, s, idx] = float(wgt)
            else:
                wf[idx[1:], d, s, idx[:-1]] = float(wgt)
    return wf.astype(ml_dtypes.float8_e4m3fn)


def quantize(img):
    bits = (QBITS_BASE + np.floor(np.asarray(img, np.float32) * QBITS_SCALE)).astype(
        np.uint16
    )
    return bits.view(np.float16)


def _host_inputs(img, h, w, rpc, ncores):
    q = quantize(img)
    pad = np.full((h + 2, w + 2), np.float16(-1.0), np.float16)
    pad[1 : h + 1, 1 : w + 1] = q
    pad[0, 1 : w + 1] = q[h - 1]
    pad[1 : h + 1, 0] = q[:, w - 1]
    pad[0, 0] = q[h - 1, w - 1]

    wid = _make_wident()
    in_maps = []
    for c in range(ncores):
        in_maps.append(
            {
                "x": np.ascontiguousarray(pad[rpc * c : rpc * c + rpc + 2, :]),
                "wident": wid,
            }
        )
    return in_maps


_NC_CACHE = None


def _get_nc():
    global _NC_CACHE
    if _NC_CACHE is None:
        _NC_CACHE = _build_bass(H, W, RPC, CW)
    return _NC_CACHE


def kernel(rgb_image: np.ndarray, _trace: bool = False, _tmpdir: str | None = None):
    from concourse import bass_utils

    img = np.asarray(rgb_image, dtype=np.float32)
    assert img.shape == (H, W), img.shape
    in_maps = _host_inputs(img, H, W, RPC, NCORES)
    nc = _get_nc()
    try:
        res = bass_utils.run_bass_kernel_spmd(
            nc,
            in_maps,
            core_ids=list(range(NCORES)),
            trace=_trace,
            tmpdir=_tmpdir,
        )
    except ModuleNotFoundError:
        res = bass_utils.run_bass_kernel_spmd(
            nc, in_maps, core_ids=list(range(NCORES)), trace=False
        )
    out = np.concatenate([r["y"] for r in res.results], axis=0)
    if _trace:
        kernel.last_results = res
    return out


# revision 6
# speedup vs baseline: 1.3039x; 1.0452x over previous
"""LBP extractor on 8 Trainium2 NeuronCores — v3 (engine-balanced).

See kernel2 docstring for the core scheme (fp16 bucket quantization,
complement trick, fp8e5 odd-byte plane views, DoubleRow assembly).

v3 additions (cost-model balancing; the DVE was the v2 bottleneck):
  * UV mega-tile: both halo row copies in one SBUF tensor so one DVE
    tensor_tensor with a [(pair, 2), (1, 2049)] access pattern computes two
    planes ({A, D}) in a single op at the 2x rate.
  * Plane C is computed as Pool subtract (diff = U - V) followed by either
    - DVE tensor_scalar is_ge vs 0 (4x mode, 0/1 plane), or
    - ACT Sign(diff + 2^-15) (a +-1 plane consumed with halved DR weights
      and a -30 constant adjustment),
    chosen per tile to balance DVE vs ACT occupancy (SIGN_FRAC).
  * One 2048-wide PSUM mega-tile (4 banks) per chunk: 16 DoubleRow matmuls
    accumulate into 512-col slices; a single wide ACT copy (+bias) converts
    to uint8.
"""

import math

import numpy as np

H = 8192
W = 8192
NCORES = 8
RPC = H // NCORES

CW = 2048
SUB = 512
TRO = 127
PCW = CW + 4

QBITS_BASE = 11264
QBITS_SCALE = 20479.0 / 256.0
SIGN_EPS = 2.0**-15

A0, B0, C0, D0 = 0, PCW, 2 * PCW, 3 * PCW

# fraction of tile-units whose C-plane bit extraction runs on ACT (Sign)
SIGN_NUM, SIGN_DEN = 0, 5

# debug switches (bisect aids)
PAIR_AD = True  # paired {A, D} compare op
POOL_DIFF = True  # C via Pool subtract (else DVE is_ge directly)
WIDE_PSUM = False  # [128, 2048] 4-bank psum + single wide ACT copy
PIPE_DIFF = False  # issue the Pool subtract one unit ahead (software pipeline)
OUT_DMA_ACT = False  # issue output DMAs from the ACT queue (SP prefetches inputs)
CHUNK_DMA = True  # per-chunk input DMAs (fast pipeline start)
CPW2 = 2052  # per-chunk uv tile: U half width (V half at same offset +CPW2)
DVE_C_EVERY = 10  # every Nth unit computes C directly on DVE (Pool/DVE balance)

# DoubleRow matmul table: (rhs_plane_elem_off, rhs_col_off, delta, w0/sh0, w1/sh1)
# indices 0..3 for is_ge-C tiles; index 4 replaces 1 on sign-C tiles.
DRS = [
    (A0, 0, 2 * 1, (1, 0), (-16, 1)),  # A direct | A' derived (rhs: A[m], A[m+1])
    (C0, 0, 2 * 1, (-64, 1), (4, 0)),  # C' | C  (0/1 plane)
    (B0, 0, 2 * (D0 - B0) + 2, (2, 0), (-8, 0)),  # B | D' (rhs: B[m], D[m+1])
    (B0, 0, 2 * (D0 - B0), (-32, 1), (128, 0)),  # B' | D
    (C0, 0, 2 * 1, (-32, 1), (2, 0)),  # C' | C  (+-1 plane, halved)
]
BIAS_ISGE = 120.0  # sum of derived weights
BIAS_SIGN = 120.0 - 30.0  # C direct 4b = 2s+2; C' -64b' = -32s'-32 -> -30


def _build_bass(h, w, rpc, cw):
    import concourse.bacc as bacc
    import concourse.bass as bass
    import concourse.mybir as mybir
    from concourse.tile import TileContext

    f16 = mybir.dt.float16
    f32 = mybir.dt.float32
    fp8e4 = mybir.dt.float8e4
    fp8e5 = mybir.dt.float8e5
    u8 = mybir.dt.uint8

    pcw = cw + 4
    w2 = w + 2
    n_tiles = math.ceil(rpc / TRO)
    n_chunks = w // cw
    n_sub = cw // SUB

    nc = bacc.Bacc("TRN2", target_bir_lowering=False)
    x = nc.dram_tensor("x", [rpc + 2, w2], f16, kind="ExternalInput")
    wident = nc.dram_tensor("wident", [128, 5, 2, 128], fp8e4, kind="ExternalInput")
    y = nc.dram_tensor("y", [rpc, w], u8, kind="ExternalOutput")

    def rap(base_ap, elem_off, dims):
        return bass.AP(
            tensor=base_ap.tensor, offset=base_ap.offset + elem_off, ap=dims
        )

    with TileContext(nc) as tc:
        with (
            tc.tile_pool(name="const", bufs=1) as cpool,
            tc.tile_pool(name="img", bufs=6) as ipool,
            tc.tile_pool(name="diff", bufs=3) as dpool,
            tc.tile_pool(name="planes", bufs=3) as ppool,
            tc.tile_pool(name="outb", bufs=3) as opool,
            tc.tile_pool(name="psum", bufs=2 if WIDE_PSUM else 8, space="PSUM") as qpool,
        ):
            wt = cpool.tile([128, 5, 2, 128], fp8e4)
            nc.sync.dma_start(wt[:, :, :, :], wident[:, :, :, :])
            eps_t = cpool.tile([128, 1], f32)
            nc.vector.memset(eps_t[:, :], SIGN_EPS)

            units = [(t, q) for t in range(n_tiles) for q in range(n_chunks)]
            if len(units) > n_chunks:
                # run the cheap ragged last row-tile first: its tiny DMAs fill
                # the pipe instantly and the tail then ends on a regular unit
                units = units[-n_chunks:] + units[:-n_chunks]
            uv_tiles = {}
            df_tiles = {}
            cpw2 = CPW2

            def tile_rows(t):
                r0 = t * TRO
                nrows = min(TRO, rpc - r0)
                return r0, nrows, nrows + 1

            def ensure_uv(t, q):
                if (t, q) not in uv_tiles:
                    r0, _nrows, k = tile_rows(t)
                    c0 = q * cw
                    cwd = min(cpw2, w2 - c0)
                    uv = ipool.tile(
                        [128, 2 * cpw2], f16, tag="uv", name=f"uv{t}_{q}"
                    )
                    nc.sync.dma_start(
                        uv[0:k, 0:cwd], x[r0 : r0 + k, c0 : c0 + cwd]
                    )
                    nc.sync.dma_start(
                        uv[0:k, cpw2 : cpw2 + cwd],
                        x[r0 + 1 : r0 + 1 + k, c0 : c0 + cwd],
                    )
                    uv_tiles[(t, q)] = uv
                return uv_tiles[(t, q)]

            def issue_sub(u):
                if not POOL_DIFF or u >= len(units):
                    return
                if DVE_C_EVERY and (u % DVE_C_EVERY) == (DVE_C_EVERY - 1):
                    return  # this unit's C runs directly on DVE
                t, q = units[u]
                _r0, _nrows, k = tile_rows(t)
                uva = ensure_uv(t, q)[:, :]
                pstr = uva.ap[0][0]
                df = dpool.tile([128, pcw], f16, tag="df", name=f"df{u}")
                nc.gpsimd.tensor_tensor(
                    out=df[0:k, 0 : cw + 1],
                    in0=rap(uva, 1, [[pstr, k], [1, cw + 1]]),
                    in1=rap(uva, cpw2, [[pstr, k], [1, cw + 1]]),
                    op=mybir.AluOpType.subtract,
                )
                df_tiles[u] = df

            if PIPE_DIFF:
                issue_sub(0)
            for unit, (t, q) in enumerate(units):
                r0, nrows, k = tile_rows(t)
                uv = ensure_uv(t, q)
                uva = uv[:, :]
                pstr = uva.ap[0][0]
                if PIPE_DIFF:
                    issue_sub(unit + 1)
                else:
                    issue_sub(unit)
                cb = q * cw
                use_sign = (unit % SIGN_DEN) < SIGN_NUM
                if True:
                    pl = ppool.tile([128, 4 * pcw], f16, tag="pl")
                    pla = pl[:, :]
                    ppstr = pla.ap[0][0]
                    # pair op {A, D}: in0 = (U[m], V[m]), in1 = V[m+1] x2
                    if PAIR_AD:
                        nc.vector.tensor_tensor(
                            out=rap(pla, A0, [[ppstr, k], [D0 - A0, 2], [1, cw + 1]]),
                            in0=rap(uva, 0, [[pstr, k], [cpw2, 2], [1, cw + 1]]),
                            in1=rap(
                                uva, cpw2 + 1, [[pstr, k], [0, 2], [1, cw + 1]]
                            ),
                            op=mybir.AluOpType.is_ge,
                        )
                    else:
                        nc.vector.tensor_tensor(
                            out=pl[0:k, A0 : A0 + cw + 1],
                            in0=rap(uva, 0, [[pstr, k], [1, cw + 1]]),
                            in1=rap(uva, cpw2 + 1, [[pstr, k], [1, cw + 1]]),
                            op=mybir.AluOpType.is_ge,
                        )
                        nc.vector.tensor_tensor(
                            out=pl[0:k, D0 : D0 + cw + 1],
                            in0=rap(uva, cpw2, [[pstr, k], [1, cw + 1]]),
                            in1=rap(uva, cpw2 + 1, [[pstr, k], [1, cw + 1]]),
                            op=mybir.AluOpType.is_ge,
                        )
                    # B = U[m+1] >= V[m+1]
                    nc.vector.tensor_tensor(
                        out=pl[0:k, B0 : B0 + cw + 1],
                        in0=rap(uva, 1, [[pstr, k], [1, cw + 1]]),
                        in1=rap(uva, cpw2 + 1, [[pstr, k], [1, cw + 1]]),
                        op=mybir.AluOpType.is_ge,
                    )
                    # C: Pool diff = U[m+1] - V[m] (issued earlier), bit-extract
                    if POOL_DIFF and unit not in df_tiles and DVE_C_EVERY:
                        use_sign = False
                        nc.vector.tensor_tensor(
                            out=pl[0:k, C0 : C0 + cw + 1],
                            in0=rap(uva, 1, [[pstr, k], [1, cw + 1]]),
                            in1=rap(uva, cpw2, [[pstr, k], [1, cw + 1]]),
                            op=mybir.AluOpType.is_ge,
                        )
                    elif POOL_DIFF:
                        df = df_tiles.pop(unit)
                        if use_sign:
                            nc.scalar.sign(
                                pl[0:k, C0 : C0 + cw + 1],
                                df[0:k, 0 : cw + 1],
                                bias=eps_t[0:k, :],
                            )
                        else:
                            nc.vector.tensor_scalar(
                                out=pl[0:k, C0 : C0 + cw + 1],
                                in0=df[0:k, 0 : cw + 1],
                                scalar1=0.0,
                                scalar2=None,
                                op0=mybir.AluOpType.is_ge,
                            )
                    else:
                        use_sign = False
                        nc.vector.tensor_tensor(
                            out=pl[0:k, C0 : C0 + cw + 1],
                            in0=rap(uva, 1, [[pstr, k], [1, cw + 1]]),
                            in1=rap(uva, cpw2, [[pstr, k], [1, cw + 1]]),
                            op=mybir.AluOpType.is_ge,
                        )
                    pl8 = pla.bitcast(fp8e5)
                    p8str = pl8.ap[0][0]
                    ou = opool.tile([128, cw], u8, tag="ou")
                    bias = BIAS_SIGN if use_sign else BIAS_ISGE
                    pss = []
                    for sq in range(n_sub):
                        if WIDE_PSUM:
                            if sq == 0:
                                ps = qpool.tile([128, cw], f32, tag="ps")
                                pss.append(ps)
                            psv = ps[:, sq * SUB : (sq + 1) * SUB]
                        else:
                            ps = qpool.tile([128, SUB], f32, tag="ps")
                            pss.append(ps)
                            psv = ps[:, :]
                        for j in range(4):
                            d = 4 if (use_sign and j == 1) else j
                            p0, coff, delta, _s0, _s1 = DRS[d]
                            rhs = rap(
                                pl8,
                                2 * (p0 + coff + SUB * sq) + 1,
                                [[p8str, k], [delta, 2], [2, SUB]],
                            )
                            nc.tensor.matmul(
                                psv,
                                lhsT=wt[0:k, d, :, :],
                                rhs=rhs,
                                start=(j == 0),
                                stop=(j == 3),
                                perf_mode=mybir.MatmulPerfMode.DoubleRow,
                                skip_group_check=True,
                            )
                    if WIDE_PSUM:
                        nc.scalar.activation(
                            ou[0:nrows, :],
                            pss[0][0:nrows, :],
                            mybir.ActivationFunctionType.Copy,
                            bias=bias,
                        )
                    else:
                        for sq in range(n_sub):
                            nc.scalar.activation(
                                ou[0:nrows, sq * SUB : (sq + 1) * SUB],
                                pss[sq][0:nrows, :],
                                mybir.ActivationFunctionType.Copy,
                                bias=bias,
                            )
                    out_eng = nc.scalar if OUT_DMA_ACT else nc.sync
                    out_eng.dma_start(
                        y[r0 : r0 + nrows, cb : cb + cw], ou[0:nrows, :]
                    )

    nc.compile()
    return nc


def _make_wident():
    import ml_dtypes

    wf = np.zeros((128, 5, 2, 128), np.float32)
    idx = np.arange(128)
    for d, (_p0, _c, _delta, s0, s1) in enumerate(DRS):
        for s, (wgt, sh) in enumerate((s0, s1)):
            if sh == 0:
                wf[idx, d, s, idx] = float(wgt)
            else:
                wf[idx[1:], d, s, idx[:-1]] = float(wgt)
    return wf.astype(ml_dtypes.float8_e4m3fn)


def quantize(img):
    bits = (QBITS_BASE + np.floor(np.asarray(img, np.float32) * QBITS_SCALE)).astype(
        np.uint16
    )
    return bits.view(np.float16)


def _host_inputs(img, h, w, rpc, ncores):
    q = quantize(img)
    pad = np.full((h + 2, w + 2), np.float16(-1.0), np.float16)
    pad[1 : h + 1, 1 : w + 1] = q
    pad[0, 1 : w + 1] = q[h - 1]
    pad[1 : h + 1, 0] = q[:, w - 1]
    pad[0, 0] = q[h - 1, w - 1]

    wid = _make_wident()
    in_maps = []
    for c in range(ncores):
        in_maps.append(
            {
                "x": np.ascontiguousarray(pad[rpc * c : rpc * c + rpc + 2, :]),
                "wident": wid,
            }
        )
    return in_maps


_NC_CACHE = None


def _get_nc():
    global _NC_CACHE
    if _NC_CACHE is None:
        _NC_CACHE = _build_bass(H, W, RPC, CW)
    return _NC_CACHE


def kernel(rgb_image: np.ndarray, _trace: bool = False, _tmpdir: str | None = None):
    from concourse import bass_utils

    img = np.asarray(rgb_image, dtype=np.float32)
    assert img.shape == (H, W), img.shape
    in_maps = _host_inputs(img, H, W, RPC, NCORES)
    nc = _get_nc()
    try:
        res = bass_utils.run_bass_kernel_spmd(
            nc,
            in_maps,
            core_ids=list(range(NCORES)),
            trace=_trace,
            tmpdir=_tmpdir,
        )
    except ModuleNotFoundError:
        res = bass_utils.run_bass_kernel_spmd(
            nc, in_maps, core_ids=list(range(NCORES)), trace=False
        )
    out = np.concatenate([r["y"] for r in res.results], axis=0)
    if _trace:
        kernel.last_results = res
    return out
